# revision 6
# baseline (speedup 1.0000x reference)
"""Bahdanau-attention kernel for Trainium2 (8 NeuronCores, data-parallel over batch).

Computation (per batch b):
    enc_proj = h_enc @ W1.T + b1          # (L, D)   -- the big matmul
    dec_proj = h_dec @ W2.T + b2          # (D,)
    h        = tanh(enc_proj + dec_proj)  # (L, D)
    scores   = h @ V (+ bv)               # (L,)  -- bv cancels in softmax, dropped
    attn     = softmax(scores)            # no-max softmax: scores ~ N(0,0.4), exp safe
    ctx      = attn @ enc_proj            # (D,)

v2 design vs v1:
  - h_enc shipped host-side pre-transposed + fp16 (d on partitions): no on-device
    cast-DMA, no xbar transpose.
  - dec_proj + b1 + b2 computed on host (bias prep).
  - ctx via associativity: ctx = (attn @ h_enc) @ W1.T + b1; enc_proj never
    materialized (no evacuation). r = exp(scores) @ h_enc on DVE; the tiny
    (r/Z) @ W1.T matmul runs once per core with all 4 batches packed into M=4.
  - Softmax/r split per 512-wide l-quarter so the last half's DVE reduction
    chain (the kernel tail) starts as soon as the first scores window is done.
"""

import numpy as np

B, L, D = 32, 2048, 1024
NCORES = 8
NB = B // NCORES  # batches per core
P = 128
NCH = D // P      # 8 chunks of the d/e dimension
NH = 2            # l-halves per batch
LH = L // NH      # 1024
NQ = 2 * NH       # l-quarters (512) per batch

_cache = {}


def _build():
    import concourse.tile as tile
    from concourse import bacc, mybir
    from concourse.bass import ts, ds
    from contextlib import ExitStack

    FP16 = mybir.dt.float16
    FP32 = mybir.dt.float32
    Alu = mybir.AluOpType
    Act = mybir.ActivationFunctionType
    X = mybir.AxisListType.X

    nc = bacc.Bacc("TRN2", name="bahdanau_attn_v2")

    # hq[b, q, c, l] = fp16(h_enc[b, l, c*128+q])  (host pre-transposed)
    hq_d = nc.dram_tensor("hq", [NB, P, NCH, L], FP16, kind="ExternalInput")
    w1t = nc.dram_tensor("w1t", [NCH, P, D], FP16, kind="ExternalInput")  # [dchunk, dpart, e]
    vrep_d = nc.dram_tensor("vrep", [P, NCH, P], FP16, kind="ExternalInput")
    bias_d = nc.dram_tensor("biasd", [P, NCH, NB], FP32, kind="ExternalInput")
    b1r_d = nc.dram_tensor("b1r", [NB, D], FP32, kind="ExternalInput")
    out = nc.dram_tensor("ctx_out", [NB, D], FP32, kind="ExternalOutput")

    with tile.TileContext(nc) as tc, ExitStack() as ctx:
        wp = ctx.enter_context(tc.tile_pool(name="weights", bufs=1))
        ld = ctx.enter_context(tc.tile_pool(name="loads", bufs=3))
        hp = ctx.enter_context(tc.tile_pool(name="htan", bufs=3))
        xp = ctx.enter_context(tc.tile_pool(name="exps", bufs=2))
        sp = ctx.enter_context(tc.tile_pool(name="scratch", bufs=2))
        fin = ctx.enter_context(tc.tile_pool(name="final", bufs=2))
        psA = ctx.enter_context(tc.tile_pool(name="psA", bufs=2, space="PSUM"))
        psS = ctx.enter_context(tc.tile_pool(name="psS", bufs=1, space="PSUM"))
        psF = ctx.enter_context(tc.tile_pool(name="psF", bufs=1, space="PSUM"))

        # ---- prologue: weights / constants ----
        w1_sb = [wp.tile([P, D], FP16, tag=f"w1_{d}", name=f"w1_{d}") for d in range(NCH)]
        for d in range(NCH):
            nc.scalar.dma_start(w1_sb[d], w1t[d])
        vrep = wp.tile([P, NCH, P], FP16)
        nc.scalar.dma_start(vrep, vrep_d[:])
        bias_sb = wp.tile([P, NCH, NB], FP32)
        nc.scalar.dma_start(bias_sb, bias_d[:])
        b1r_sb = wp.tile([NB, D], FP32)
        nc.scalar.dma_start(b1r_sb, b1r_d[:])

        # r16_all[:, dc, b] = fp16((exp@h_enc / Z)[b, dc*128+q]) -- persists all batches
        r16_all = wp.tile([P, NCH, NB], FP16, tag="r16")

        # ---- main loop over batches ----
        for b in range(NB):
            exp_rep = xp.tile([P, L], FP16, tag="exp")  # exp(scores), replicated rows
            zsl = fin.tile([P, NQ], FP32, tag="zsl")    # per-quarter sum of exp
            r_sl = fin.tile([P, NCH, NQ], FP32, tag="rsl")

            for h in range(NH):
                hq_t = ld.tile([P, NCH, LH], FP16, tag="hq")
                # two DMAs (one per 512-l window) so the first matmuls start early
                for g in range(2):
                    nc.sync.dma_start(
                        hq_t[:, :, ds(g * 512, 512)],
                        hq_d[b, :, :, ds(h * LH + g * 512, 512)],
                    )

                ps_sc = psS.tile([P, LH], FP32, tag="sc")
                prev = None  # software pipeline: scores(c-1) issue between main(c)
                for c in range(NCH):
                    ps = psA.tile([P, LH], FP32, tag="mm")
                    for g in range(2):
                        for dc in range(NCH):
                            nc.tensor.matmul(
                                ps[:, ts(g, 512)],
                                lhsT=w1_sb[dc][:, ts(c, P)],
                                rhs=hq_t[:, dc, ds(g * 512, 512)],
                                start=(dc == 0),
                                stop=(dc == NCH - 1),
                            )
                    # tanh(enc_projT + dec_proj + b1 + b2), fused bias on ACT
                    ht = hp.tile([P, LH], FP16, tag="ht")
                    nc.scalar.activation(ht, ps, Act.Tanh, bias=bias_sb[:, c, b : b + 1])
                    if prev is not None:
                        pc, pht = prev
                        for g in range(2):
                            nc.tensor.matmul(
                                ps_sc[:, ts(g, 512)], lhsT=vrep[:, pc, :],
                                rhs=pht[:, ts(g, 512)],
                                start=(pc == 0), stop=False,
                            )
                    prev = (c, ht)
                pc, pht = prev
                # last chunk's scores; then per-512-window: exp + r partials, so
                # the DVE chain overlaps the remaining PE work
                for g in range(2):
                    nc.tensor.matmul(
                        ps_sc[:, ts(g, 512)], lhsT=vrep[:, pc, :],
                        rhs=pht[:, ts(g, 512)],
                        start=False, stop=True,
                    )
                    qq = 2 * h + g
                    nc.scalar.activation(
                        exp_rep[:, ds(qq * 512, 512)], ps_sc[:, ts(g, 512)], Act.Exp,
                        accum_out=zsl[:, qq : qq + 1],
                    )
                    with nc.allow_low_precision("fp16 product scratch; |e*h| < 40"):
                        for c in range(NCH):
                            scr = sp.tile([P, 512], FP16, tag="scr")
                            nc.vector.tensor_tensor(
                                scr, hq_t[:, c, ds(g * 512, 512)],
                                exp_rep[:, ds(qq * 512, 512)], Alu.mult,
                            )
                            nc.vector.tensor_reduce(
                                r_sl[:, c, qq : qq + 1], scr, axis=X, op=Alu.add
                            )

            # finalize batch: r16 = (r_q0+..+r_q3) / Z
            zsum = fin.tile([P, 1], FP32, tag="zsum")
            nc.vector.tensor_reduce(zsum, zsl, axis=X, op=Alu.add)
            recip = fin.tile([P, 1], FP32, tag="recip")
            nc.vector.reciprocal(recip, zsum)
            rsum = fin.tile([P, NCH], FP32, tag="rsum")
            nc.vector.tensor_reduce(rsum, r_sl, axis=X, op=Alu.add)
            nc.vector.tensor_scalar(
                out=r16_all[:, :, b], in0=rsum, scalar1=recip, scalar2=None, op0=Alu.mult
            )

        # ---- core end: ctx = r16_all.T @ W1.T + b1, all batches packed M=4 ----
        psf = psF.tile([NB, D], FP32, tag="f")
        for w in range(2):
            for dc in range(NCH):
                nc.tensor.matmul(
                    psf[:, ts(w, 512)],
                    lhsT=r16_all[:, dc, :],
                    rhs=w1_sb[dc][:, ts(w, 512)],
                    start=(dc == 0),
                    stop=(dc == NCH - 1),
                )
        ctx_sb = fin.tile([NB, D], FP32, tag="ctx")
        nc.vector.tensor_tensor(ctx_sb, psf, b1r_sb, Alu.add)
        nc.scalar.dma_start(out[:], ctx_sb)

    nc.finalize()
    return nc


def kernel(h_enc, h_dec, W1, b1, W2, b2, V, bv):
    from concourse.bass_utils import run_bass_kernel_spmd

    h_enc = np.asarray(h_enc, dtype=np.float32)
    h_dec = np.asarray(h_dec, dtype=np.float32)
    W1 = np.asarray(W1, dtype=np.float32)
    b1 = np.asarray(b1, dtype=np.float32)
    W2 = np.asarray(W2, dtype=np.float32)
    b2 = np.asarray(b2, dtype=np.float32)
    V = np.asarray(V, dtype=np.float32)

    if "nc" not in _cache:
        _cache["nc"] = _build()
    nc = _cache["nc"]

    f16 = np.float16
    w1t = np.ascontiguousarray(W1.T).reshape(NCH, P, D).astype(f16)
    vt = V.reshape(NCH, P).T  # [P, NCH]
    vrep = np.ascontiguousarray(np.broadcast_to(vt[:, :, None], (P, NCH, P))).astype(f16)
    b1r = np.ascontiguousarray(np.broadcast_to(b1[None, :], (NB, D))).astype(np.float32)
    dec_all = (h_dec @ W2.T + b1 + b2).astype(np.float32)  # [B, D]

    h16 = h_enc.astype(f16)  # [B, L, D]

    in_maps = []
    for core in range(NCORES):
        sl = slice(core * NB, (core + 1) * NB)
        hq = np.ascontiguousarray(
            h16[sl].reshape(NB, L, NCH, P).transpose(0, 3, 2, 1)
        )
        biasd = np.ascontiguousarray(
            dec_all[sl].T.reshape(NCH, P, NB).transpose(1, 0, 2)
        )
        in_maps.append(
            {"hq": hq, "w1t": w1t, "vrep": vrep, "biasd": biasd, "b1r": b1r}
        )

    res = run_bass_kernel_spmd(nc, in_maps, core_ids=list(range(NCORES)))
    globals()["LAST_RES"] = res
    outs = [res.results[core]["ctx_out"] for core in range(NCORES)]
    return np.concatenate(outs, axis=0).astype(np.float32)


# revision 7
# speedup vs baseline: 1.4410x; 1.4410x over previous
"""v3: v2.1 + half the contraction (d-chunks 0-3) in fp8e4 DoubleRow.

The enc_proj matmul feeds only tanh->scores->softmax (ctx goes through the
exact fp16 r-path), so it tolerates reduced precision. e4m3 for BOTH operands
on half of K, with host-side coordinated rounding of W8 that zeroes the
dominant error functional u_d = sum_e V_e * dW[e,d] (the systematic part that
softmax can't average away). Sim rel-err: 0.0131 (gate 2e-2).

Scale plumbing: both the fp8 W (e4m3 needs W*512 to stay out of subnormals)
and the fp16 W tiles hold W.T*512, so the two parts share one PSUM
accumulation; tanh applies scale=1/512, and the final ctx matmul descales by
1/512 on DVE.
"""

import numpy as np

B, L, D = 32, 2048, 1024
NCORES = 8
NB = B // NCORES
P = 128
NCH = D // P
NPAIR = 2        # DoubleRow pairs covering d-chunks 0-3
NF16 = NCH - 2 * NPAIR  # d-chunks in fp16 (4-7)
NH = 2
LH = L // NH
NQ = 2 * NH
WSC = 512.0

_cache = {}


def _build():
    import concourse.tile as tile
    from concourse import bacc, mybir
    from concourse.bass import ts, ds
    from contextlib import ExitStack

    FP16 = mybir.dt.float16
    FP32 = mybir.dt.float32
    FP8 = mybir.dt.float8e4
    Alu = mybir.AluOpType
    Act = mybir.ActivationFunctionType
    X = mybir.AxisListType.X
    DR = mybir.MatmulPerfMode.DoubleRow

    nc = bacc.Bacc("TRN2", name="bahdanau_attn_v3")

    hq_d = nc.dram_tensor("hq", [NB, P, NCH, L], FP16, kind="ExternalInput")
    w1t = nc.dram_tensor("w1t", [NCH, P, D], FP16, kind="ExternalInput")  # W1.T * 512
    w18_d = nc.dram_tensor("w18", [P, NPAIR, 2, NCH, P], FP8, kind="ExternalInput")
    vrep_d = nc.dram_tensor("vrep", [P, NCH, P], FP16, kind="ExternalInput")
    bias_d = nc.dram_tensor("biasd", [P, NCH, NB], FP32, kind="ExternalInput")
    b1r_d = nc.dram_tensor("b1r", [NB, D], FP32, kind="ExternalInput")
    out = nc.dram_tensor("ctx_out", [NB, D], FP32, kind="ExternalOutput")

    with tile.TileContext(nc) as tc, ExitStack() as ctx:
        wp = ctx.enter_context(tc.tile_pool(name="weights", bufs=1))
        ld = ctx.enter_context(tc.tile_pool(name="loads", bufs=3))
        l8 = ctx.enter_context(tc.tile_pool(name="loads8", bufs=3))
        hp = ctx.enter_context(tc.tile_pool(name="htan", bufs=3))
        xp = ctx.enter_context(tc.tile_pool(name="exps", bufs=2))
        sp = ctx.enter_context(tc.tile_pool(name="scratch", bufs=4))
        fin = ctx.enter_context(tc.tile_pool(name="final", bufs=2))
        psA = ctx.enter_context(tc.tile_pool(name="psA", bufs=2, space="PSUM"))
        psS = ctx.enter_context(tc.tile_pool(name="psS", bufs=1, space="PSUM"))
        psF = ctx.enter_context(tc.tile_pool(name="psF", bufs=1, space="PSUM"))

        # ---- prologue ----
        w1_sb = [wp.tile([P, D], FP16, tag=f"w1_{d}", name=f"w1_{d}") for d in range(NCH)]
        for d in range(2 * NPAIR, NCH):  # fp16 matmul tiles needed first
            nc.scalar.dma_start(w1_sb[d], w1t[d])
        w18_sb = wp.tile([P, NPAIR, 2, NCH, P], FP8, tag="w18")
        nc.scalar.dma_start(w18_sb, w18_d[:])
        vrep = wp.tile([P, NCH, P], FP16)
        nc.scalar.dma_start(vrep, vrep_d[:])
        bias_sb = wp.tile([P, NCH, NB], FP32)
        nc.scalar.dma_start(bias_sb, bias_d[:])
        b1r_sb = wp.tile([NB, D], FP32)
        nc.scalar.dma_start(b1r_sb, b1r_d[:])
        for d in range(0, 2 * NPAIR):  # only needed by the final ctx matmul
            nc.scalar.dma_start(w1_sb[d], w1t[d])

        r16_all = wp.tile([P, NCH, NB], FP16, tag="r16")

        # ---- main loop ----
        for b in range(NB):
            exp_rep = xp.tile([P, L], FP16, tag="exp")
            zsl = fin.tile([P, NQ], FP32, tag="zsl")
            r_sl = fin.tile([P, NCH, NQ], FP32, tag="rsl")

            for h in range(NH):
                hq_t = ld.tile([P, NCH, LH], FP16, tag="hq")
                # fp8 copy of d-chunks 0-3, pair-plane layout for DoubleRow;
                # per-512-window DMAs + casts so the first matmuls start early
                h8_t = l8.tile([P, NPAIR, 2, LH], FP8, tag="h8")
                for g in range(2):
                    nc.sync.dma_start(
                        hq_t[:, :, ds(g * 512, 512)],
                        hq_d[b, :, :, ds(h * LH + g * 512, 512)],
                    )
                    for j in range(NPAIR):
                        for i in range(2):
                            nc.vector.tensor_copy(
                                h8_t[:, j, i, ds(g * 512, 512)],
                                hq_t[:, 2 * j + i, ds(g * 512, 512)],
                            )

                ps_sc = psS.tile([P, LH], FP32, tag="sc")
                prev = None
                for c in range(NCH):
                    ps = psA.tile([P, LH], FP32, tag="mm")
                    for g in range(2):
                        # fp16 chunks first: they only need the hq DMA, not the
                        # DVE cast, so the PE starts sooner at kernel startup
                        for dc in range(2 * NPAIR, NCH):
                            nc.tensor.matmul(
                                ps[:, ts(g, 512)],
                                lhsT=w1_sb[dc][:, ts(c, P)],
                                rhs=hq_t[:, dc, ds(g * 512, 512)],
                                start=(dc == 2 * NPAIR),
                                stop=False,
                            )
                        for j in range(NPAIR):
                            nc.tensor.matmul(
                                ps[:, ts(g, 512)],
                                lhsT=w18_sb[:, j, :, c, :],
                                rhs=h8_t[:, j, :, ds(g * 512, 512)],
                                start=False,
                                stop=(j == NPAIR - 1),
                                perf_mode=DR,
                            )
                    ht = hp.tile([P, LH], FP16, tag="ht")
                    nc.scalar.activation(
                        ht, ps, Act.Tanh, bias=bias_sb[:, c, b : b + 1], scale=1.0 / WSC
                    )
                    if prev is not None:
                        pc, pht = prev
                        for g in range(2):
                            nc.tensor.matmul(
                                ps_sc[:, ts(g, 512)], lhsT=vrep[:, pc, :],
                                rhs=pht[:, ts(g, 512)],
                                start=(pc == 0), stop=False,
                            )
                    prev = (c, ht)
                pc, pht = prev
                for g in range(2):
                    nc.tensor.matmul(
                        ps_sc[:, ts(g, 512)], lhsT=vrep[:, pc, :],
                        rhs=pht[:, ts(g, 512)],
                        start=False, stop=True,
                    )
                    qq = 2 * h + g
                    nc.scalar.activation(
                        exp_rep[:, ds(qq * 512, 512)], ps_sc[:, ts(g, 512)], Act.Exp,
                        accum_out=zsl[:, qq : qq + 1],
                    )
                    # r partials: product on DVE; reduce on DVE mid-kernel but
                    # on ACT (Copy + accum_out) for the last half, where the
                    # reduce chain is the kernel tail and ACT sits idle
                    tailq = b == NB - 1 and h == NH - 1
                    with nc.allow_low_precision("fp16 product scratch; |e*h| < 40"):
                        for c in range(NCH):
                            scr = sp.tile([P, 512], FP16, tag="scr")
                            nc.vector.tensor_tensor(
                                scr, hq_t[:, c, ds(g * 512, 512)],
                                exp_rep[:, ds(qq * 512, 512)], Alu.mult,
                            )
                            if tailq:
                                scr2 = sp.tile([P, 512], FP16, tag="scr2")
                                nc.scalar.activation(
                                    scr2, scr, Act.Copy,
                                    accum_out=r_sl[:, c, qq : qq + 1],
                                )
                            else:
                                nc.vector.tensor_reduce(
                                    r_sl[:, c, qq : qq + 1], scr, axis=X, op=Alu.add
                                )

            zsum = fin.tile([P, 1], FP32, tag="zsum")
            nc.vector.tensor_reduce(zsum, zsl, axis=X, op=Alu.add)
            recip = fin.tile([P, 1], FP32, tag="recip")
            nc.vector.reciprocal(recip, zsum)
            rsum = fin.tile([P, NCH], FP32, tag="rsum")
            nc.vector.tensor_reduce(rsum, r_sl, axis=X, op=Alu.add)
            nc.vector.tensor_scalar(
                out=r16_all[:, :, b], in0=rsum, scalar1=recip, scalar2=None, op0=Alu.mult
            )

        # ---- core end: ctx = (r16_all.T @ (W1.T*512)) / 512 + b1 ----
        psf = psF.tile([NB, D], FP32, tag="f")
        for w in range(2):
            for dc in range(NCH):
                nc.tensor.matmul(
                    psf[:, ts(w, 512)],
                    lhsT=r16_all[:, dc, :],
                    rhs=w1_sb[dc][:, ts(w, 512)],
                    start=(dc == 0),
                    stop=(dc == NCH - 1),
                )
        ctxs = fin.tile([NB, D], FP32, tag="ctxs")
        nc.vector.tensor_scalar(
            out=ctxs, in0=psf, scalar1=1.0 / WSC, scalar2=None, op0=Alu.mult
        )
        ctx_sb = fin.tile([NB, D], FP32, tag="ctx")
        nc.vector.tensor_tensor(ctx_sb, ctxs, b1r_sb, Alu.add)
        nc.scalar.dma_start(out[:], ctx_sb)

    nc.finalize()
    return nc


def _coord_round_w8(W1, V):
    """e4m3-quantize W1.T*512 for d-chunks 0-3 with greedy coordinated rounding
    that minimizes u_d = sum_e V_e * dW[e, d]."""
    import ml_dtypes

    f8 = ml_dtypes.float8_e4m3
    K8 = 2 * NPAIR * P  # 512 d's
    WS = (W1.T[:K8] * WSC).astype(np.float64)  # [d, e]
    xr = WS.astype(f8).astype(np.float64)
    step = np.spacing(np.abs(xr).astype(f8)).astype(np.float64)
    alt = np.where(xr < WS, xr + step * 1.001, xr - step * 1.001)
    alt = alt.astype(f8).astype(np.float64)
    dn = np.minimum(xr, alt)
    up = np.maximum(xr, alt)
    exact = xr == WS
    dn = np.where(exact, xr, dn)
    up = np.where(exact, xr, up)

    order = np.argsort(-np.abs(V))
    Wc = np.empty_like(WS)
    run = np.zeros(K8)
    for e in order:
        d_dn = dn[:, e] - WS[:, e]
        d_up = up[:, e] - WS[:, e]
        cand_dn = run + V[e] * d_dn
        cand_up = run + V[e] * d_up
        pick_dn = np.abs(cand_dn) <= np.abs(cand_up)
        Wc[:, e] = np.where(pick_dn, dn[:, e], up[:, e])
        run = np.where(pick_dn, cand_dn, cand_up)
    return Wc.astype(f8)  # [512, 1024] e4m3, scaled by 512


def _prep_w18(W1, V):
    """Arrange coordinated-rounded W8 as [q, j, i, c, e] for DoubleRow lhsT."""
    Wc = _coord_round_w8(W1, V)  # [d=512, e=1024] e4m3
    # w18[q, j, i, c, e] = Wc[(2j+i)*128+q, c*128+e]
    w = Wc.reshape(NPAIR, 2, P, NCH, P)  # [j, i, q, c, e]
    return np.ascontiguousarray(w.transpose(2, 0, 1, 3, 4))


def kernel(h_enc, h_dec, W1, b1, W2, b2, V, bv):
    from concourse.bass_utils import run_bass_kernel_spmd

    h_enc = np.asarray(h_enc, dtype=np.float32)
    h_dec = np.asarray(h_dec, dtype=np.float32)
    W1 = np.asarray(W1, dtype=np.float32)
    b1 = np.asarray(b1, dtype=np.float32)
    W2 = np.asarray(W2, dtype=np.float32)
    b2 = np.asarray(b2, dtype=np.float32)
    V = np.asarray(V, dtype=np.float32)

    if "nc" not in _cache:
        _cache["nc"] = _build()
    nc = _cache["nc"]

    f16 = np.float16
    w1t = (np.ascontiguousarray(W1.T).reshape(NCH, P, D) * WSC).astype(f16)
    w18 = _prep_w18(W1, V)
    vt = V.reshape(NCH, P).T
    vrep = np.ascontiguousarray(np.broadcast_to(vt[:, :, None], (P, NCH, P))).astype(f16)
    b1r = np.ascontiguousarray(np.broadcast_to(b1[None, :], (NB, D))).astype(np.float32)
    dec_all = (h_dec @ W2.T + b1 + b2).astype(np.float32)

    h16 = h_enc.astype(f16)

    in_maps = []
    for core in range(NCORES):
        sl = slice(core * NB, (core + 1) * NB)
        hq = np.ascontiguousarray(h16[sl].reshape(NB, L, NCH, P).transpose(0, 3, 2, 1))
        biasd = np.ascontiguousarray(dec_all[sl].T.reshape(NCH, P, NB).transpose(1, 0, 2))
        in_maps.append(
            {"hq": hq, "w1t": w1t, "w18": w18, "vrep": vrep, "biasd": biasd, "b1r": b1r}
        )

    res = run_bass_kernel_spmd(nc, in_maps, core_ids=list(range(NCORES)))
    globals()["LAST_RES"] = res
    outs = [res.results[core]["ctx_out"] for core in range(NCORES)]
    return np.concatenate(outs, axis=0).astype(np.float32)
